# revision 7
# baseline (speedup 1.0000x reference)
"""Causal self-attention (B=2, S=2048, D=1024, H=16, hd=64) on 8 TRN2 cores.

Sharding: data-parallel over batch (2) x tensor-parallel over heads (16/4=4
heads per core).  Each core computes qkv projections for its 4 heads, RoPE,
causal flash-attention, and a partial output projection (row-parallel over
the 256 local attention channels).  Host sums the 4 partials per batch.

Numerics: fp16 operands everywhere on the PE (1 cycle/row), fp32 PSUM
accumulation.  Softmax without max-subtraction (scores ~ N(0,1), exp is
safe) so the denominator comes from an all-ones column appended to V.

Layout tricks:
 - Scores are computed transposed (S^T[k, q]) so probabilities feed the
   PV matmul directly as the stationary operand - no PE transposes of P.
 - RoPE pairs are host-permuted to a half-split layout (rotation partner
   lives 32 partitions away); the partner tensor is made with 4 SBUF->SBUF
   partition-swap DMAs and the sign lives in the host-built sin table.
"""

import math
import os

import numpy as np

D_MODEL = 1024
NUM_HEADS = 16
HEAD_DIM = 64
S = 2048
B = 2
N_CORES = 8
HPC = 4  # heads per core
EV = HPC * HEAD_DIM  # 256 local attention channels
ROPE_THETA = 10000.0
KB = S // 128  # 16 key blocks
F16 = np.float16

_PROGRAM = None  # (nc, input names) cache


# --------------------------------------------------------------------------
# host-side input prep
# --------------------------------------------------------------------------

def _rope_rows(base):
    """Row indices of one head's projection in half-split (permuted) order."""
    return [base + 2 * i for i in range(32)] + [base + 2 * i + 1 for i in range(32)]


def _rope_tables():
    inv_freq = 1.0 / (ROPE_THETA ** (np.arange(0, HEAD_DIM, 2, dtype=np.float64) / HEAD_DIM))
    freqs = np.outer(np.arange(S, dtype=np.float64), inv_freq)  # [S, 32]
    cos_t = np.cos(freqs).T  # [32, S]
    sin_t = np.sin(freqs).T
    cos_full = np.tile(cos_t, (4, 1)).astype(F16)  # [128, S]
    sin_full = np.tile(np.concatenate([-sin_t, sin_t], axis=0), (2, 1)).astype(F16)
    return cos_full, sin_full


def _prep_core_inputs(x, w_qkv, w_out, core):
    b, hg = core // 4, core % 4
    heads = [HPC * hg + j for j in range(HPC)]

    xT = np.ascontiguousarray(x[b].T).astype(F16)  # [1024, 2048]

    row_order = []
    for base in (0, D_MODEL):  # q rows then k rows
        for h in heads:
            row_order += _rope_rows(base + h * HEAD_DIM)
    wqk = np.ascontiguousarray(w_qkv[row_order].T).astype(F16)  # [1024, 512]

    v_rows = [2 * D_MODEL + h * HEAD_DIM + j for h in heads for j in range(HEAD_DIM)]
    wv = np.ascontiguousarray(w_qkv[v_rows].T).astype(F16)  # [1024, 256]

    out_cols = [h * HEAD_DIM + j for h in heads for j in range(HEAD_DIM)]
    wout = np.ascontiguousarray(w_out[:, out_cols].T).astype(F16)  # [256, 1024]

    cos_full, sin_full = _rope_tables()
    tri = (np.arange(128)[None, :] >= np.arange(128)[:, None]).astype(F16)  # [k,q] keep q>=k
    ident = np.eye(128, dtype=F16)

    return {
        "xT": xT,
        "wqk": wqk,
        "wv": wv,
        "wout": wout,
        "cos_t": cos_full,
        "sin_t": sin_full,
        "tri": tri,
        "ident": ident,
    }


# --------------------------------------------------------------------------
# device program
# --------------------------------------------------------------------------

def _build_body(tc, io):
    import concourse.bass as bass
    import concourse.mybir as mybir
    from contextlib import ExitStack

    f16 = mybir.dt.float16
    f32 = mybir.dt.float32
    nc = tc.nc

    xT_d, wqk_d, wv_d, wout_d = io["xT"], io["wqk"], io["wv"], io["wout"]
    cos_d, sin_d, tri_d, ident_d, out_d = (
        io["cos_t"], io["sin_t"], io["tri"], io["ident"], io["out"],
    )

    with ExitStack() as ctx:
        const = ctx.enter_context(tc.tile_pool(name="const", bufs=1))
        vpool = ctx.enter_context(tc.tile_pool(name="vpool", bufs=1))
        qkr = ctx.enter_context(tc.tile_pool(name="qkr", bufs=1))
        attn_p = ctx.enter_context(tc.tile_pool(name="attn", bufs=1))

        # ---- persistent constants -------------------------------------
        wqk_sb = [const.tile([128, 512], f16, tag=f"wqk{d}", name=f"wqk{d}") for d in range(8)]
        wv_sb = [const.tile([128, EV], f16, tag=f"wv{d}", name=f"wv{d}") for d in range(8)]
        wout_sb = [const.tile([128, 1024], f16, tag=f"wout{t}", name=f"wout{t}") for t in range(2)]
        cos_sb = const.tile([128, S], f16, tag="cos", name="cos")
        sin_sb = const.tile([128, S], f16, tag="sin", name="sin")
        tri_sb = const.tile([128, 128], f16, tag="tri", name="tri")
        id_sb = const.tile([128, 128], f16, tag="ident", name="ident")
        for d in range(8):
            nc.sync.dma_start(out=wqk_sb[d][:], in_=wqk_d[128 * d:128 * (d + 1), :])
            nc.sync.dma_start(out=wv_sb[d][:], in_=wv_d[128 * d:128 * (d + 1), :])
        for t in range(2):
            nc.sync.dma_start(out=wout_sb[t][:], in_=wout_d[128 * t:128 * (t + 1), :])
        nc.sync.dma_start(out=cos_sb[:], in_=cos_d[:])
        nc.sync.dma_start(out=sin_sb[:], in_=sin_d[:])
        nc.sync.dma_start(out=tri_sb[:], in_=tri_d[:])
        nc.sync.dma_start(out=id_sb[:], in_=ident_d[:])

        # V tiles [128 kpos, 64+1], last col = 1.0 (softmax denominator)
        v_sb = [[vpool.tile([128, HEAD_DIM + 1], f16, tag=f"v{h}_{i}", name=f"v{h}_{i}")
                 for i in range(KB)] for h in range(HPC)]
        for h in range(HPC):
            for i in range(KB):
                nc.gpsimd.memset(v_sb[h][i][:, HEAD_DIM:HEAD_DIM + 1], 1.0)

        # rotated q/k, [128 = 2 heads x 64 dims, S]
        qr_sb = [qkr.tile([128, S], f16, tag=f"qr{t}", name=f"qr{t}") for t in range(2)]
        kr_sb = [qkr.tile([128, S], f16, tag=f"kr{t}", name=f"kr{t}") for t in range(2)]

        # attn^T [128 local dims, S]
        at_sb = [attn_p.tile([128, S], f16, tag=f"at{t}", name=f"at{t}") for t in range(2)]

        with ExitStack() as phase_bc:
            px = phase_bc.enter_context(tc.tile_pool(name="px", bufs=1))
            prope = phase_bc.enter_context(tc.tile_pool(name="prope", bufs=1))
            ptmp = phase_bc.enter_context(tc.tile_pool(name="ptmp", bufs=2))
            ps_big = phase_bc.enter_context(
                tc.tile_pool(name="ps_big", bufs=4, space="PSUM"))

            xT_sb = [px.tile([128, S], f16, tag=f"x{d}", name=f"x{d}") for d in range(8)]
            for d in range(8):
                nc.sync.dma_start(out=xT_sb[d][:], in_=xT_d[128 * d:128 * (d + 1), :])

            # ---- QK projection:  psum[e=128, s=512] = sum_d W^T x^T ----
            q_sb = [prope.tile([128, S], f16, tag=f"q{t}", name=f"q{t}") for t in range(2)]
            k_sb = [prope.tile([128, S], f16, tag=f"k{t}", name=f"k{t}") for t in range(2)]
            qk_dst = q_sb + k_sb
            for t in range(4):
                for sc in range(4):
                    ps = ps_big.tile([128, 512], f32, tag="mm", name="mm")
                    for d in range(8):
                        nc.tensor.matmul(
                            ps[:],
                            lhsT=wqk_sb[d][:, 128 * t:128 * (t + 1)],
                            rhs=xT_sb[d][:, 512 * sc:512 * (sc + 1)],
                            start=(d == 0), stop=(d == 7),
                        )
                    nc.vector.tensor_copy(
                        qk_dst[t][:, 512 * sc:512 * (sc + 1)], ps[:])

            # ---- V projection:  psum[s=128, ev=256] = sum_d x^T^T Wv ----
            for sb in range(KB):
                ps = ps_big.tile([128, EV], f32, tag="mm", name="mm")
                for d in range(8):
                    nc.tensor.matmul(
                        ps[:],
                        lhsT=xT_sb[d][:, 128 * sb:128 * (sb + 1)],
                        rhs=wv_sb[d][:],
                        start=(d == 0), stop=(d == 7),
                    )
                for h in range(HPC):
                    nc.vector.tensor_copy(
                        v_sb[h][sb][:, 0:HEAD_DIM],
                        ps[:, HEAD_DIM * h:HEAD_DIM * (h + 1)])

            # ---- RoPE ------------------------------------------------
            # qr = q*cos + swap32(q)*sin_signed   (sign folded into table)
            for src, dst in zip(q_sb + k_sb, qr_sb + kr_sb):
                sw = ptmp.tile([128, S], f16, tag="sw", name="sw")
                for a, bq in ((0, 32), (32, 0), (64, 96), (96, 64)):
                    nc.sync.dma_start(out=sw[a:a + 32, :], in_=src[bq:bq + 32, :])
                t1 = ptmp.tile([128, S], f16, tag="t1", name="t1")
                nc.vector.tensor_mul(t1[:], src[:], cos_sb[:])
                t2 = ptmp.tile([128, S], f16, tag="t2", name="t2")
                nc.vector.tensor_mul(t2[:], sw[:], sin_sb[:])
                nc.vector.tensor_add(dst[:], t1[:], t2[:])

        # ---- attention + output projection ---------------------------
        with ExitStack() as phase_de:
            epool = phase_de.enter_context(tc.tile_pool(name="epool", bufs=18))
            pstage = phase_de.enter_context(tc.tile_pool(name="pstage", bufs=4))
            pnorm = phase_de.enter_context(tc.tile_pool(name="pnorm", bufs=4))
            ps_s = phase_de.enter_context(
                tc.tile_pool(name="ps_s", bufs=2, space="PSUM"))
            ps_sm = phase_de.enter_context(
                tc.tile_pool(name="ps_sm", bufs=2, space="PSUM"))

            scale = 1.0 / math.sqrt(HEAD_DIM)
            for h in range(HPC):
                tq, ro = h // 2, (h % 2) * 64
                e_tiles = []
                # scores^T + exp, k-block at a time
                for i in range(KB):
                    q_lo = 128 * i
                    et = epool.tile([128, S], f16, tag="e", name="e")
                    e_tiles.append(et)
                    for qc in range(q_lo, S, 1024):
                        qw = min(1024, S - qc)
                        ps = ps_s.tile([128, 1024], f32, tag="s", name="s")
                        for o in range(0, qw, 512):
                            w = min(512, qw - o)
                            nc.tensor.matmul(
                                ps[:, o:o + w],
                                lhsT=kr_sb[tq][ro:ro + 64, q_lo:q_lo + 128],
                                rhs=qr_sb[tq][ro:ro + 64, qc + o:qc + o + w],
                                start=True, stop=True,
                            )
                        nc.scalar.activation(
                            et[:, qc:qc + qw], ps[:, 0:qw],
                            mybir.ActivationFunctionType.Exp, scale=scale)
                    # causal mask of the diagonal block
                    nc.vector.tensor_mul(
                        et[:, q_lo:q_lo + 128], et[:, q_lo:q_lo + 128], tri_sb[:])

                # PV + normalize + transpose into attn^T
                for bq in range(KB):
                    pav = ps_sm.tile([128, HEAD_DIM + 1], f32, tag="av", name="av")
                    for i in range(bq + 1):
                        nc.tensor.matmul(
                            pav[:],
                            lhsT=e_tiles[i][:, 128 * bq:128 * (bq + 1)],
                            rhs=v_sb[h][i][:],
                            start=(i == 0), stop=(i == bq),
                        )
                    rec = pnorm.tile([128, 1], f32, tag="rec", name="rec")
                    nc.vector.reciprocal(rec[:], pav[:, HEAD_DIM:HEAD_DIM + 1])
                    ab = pnorm.tile([128, HEAD_DIM], f16, tag="ab", name="ab")
                    nc.vector.tensor_scalar_mul(ab[:], pav[:, 0:HEAD_DIM], rec[:])
                    ptr = ps_sm.tile([64, 128], f16, tag="tr", name="tr")
                    nc.tensor.transpose(ptr[:], ab[:], id_sb[:])
                    nc.vector.tensor_copy(
                        at_sb[tq][ro:ro + 64, 128 * bq:128 * (bq + 1)], ptr[:])

            # ---- output projection ----------------------------------
            for sb in range(KB):
                for ec in range(2):
                    ps = ps_s.tile([128, 512], f32, tag="s", name="s")
                    for t in range(2):
                        nc.tensor.matmul(
                            ps[:],
                            lhsT=at_sb[t][:, 128 * sb:128 * (sb + 1)],
                            rhs=wout_sb[t][:, 512 * ec:512 * (ec + 1)],
                            start=(t == 0), stop=(t == 1),
                        )
                    ot = pstage.tile([128, 512], f16, tag="o", name="o")
                    nc.vector.tensor_copy(ot[:], ps[:])
                    nc.sync.dma_start(
                        out=out_d[128 * sb:128 * (sb + 1), 512 * ec:512 * (ec + 1)],
                        in_=ot[:])


def build_program():
    global _PROGRAM
    if _PROGRAM is not None:
        return _PROGRAM
    import concourse.bacc as bacc
    import concourse.tile as tile
    import concourse.mybir as mybir

    f16 = mybir.dt.float16
    nc = bacc.Bacc("TRN2", target_bir_lowering=False, debug=False)
    io = {
        "xT": nc.dram_tensor("xT", [D_MODEL, S], f16, kind="ExternalInput").ap(),
        "wqk": nc.dram_tensor("wqk", [D_MODEL, 512], f16, kind="ExternalInput").ap(),
        "wv": nc.dram_tensor("wv", [D_MODEL, EV], f16, kind="ExternalInput").ap(),
        "wout": nc.dram_tensor("wout", [EV, D_MODEL], f16, kind="ExternalInput").ap(),
        "cos_t": nc.dram_tensor("cos_t", [128, S], f16, kind="ExternalInput").ap(),
        "sin_t": nc.dram_tensor("sin_t", [128, S], f16, kind="ExternalInput").ap(),
        "tri": nc.dram_tensor("tri", [128, 128], f16, kind="ExternalInput").ap(),
        "ident": nc.dram_tensor("ident", [128, 128], f16, kind="ExternalInput").ap(),
        "out": nc.dram_tensor("out", [S, D_MODEL], f16, kind="ExternalOutput").ap(),
    }
    with tile.TileContext(nc) as tc:
        _build_body(tc, io)
    nc.compile()
    _PROGRAM = nc
    return nc


def make_in_maps(x, w_qkv, w_out):
    return [
        {k: v for k, v in _prep_core_inputs(x, w_qkv, w_out, c).items()}
        for c in range(N_CORES)
    ]


def assemble(results):
    """results: list of 8 dicts with 'out' [S, D] fp16 -> full [B, S, D] fp32."""
    out = np.zeros((B, S, D_MODEL), dtype=np.float32)
    for c in range(N_CORES):
        out[c // 4] += results[c]["out"].astype(np.float32)
    return out


_RUNNER = None


def get_runner():
    """Persistent jitted shard_map over the 8 cores (compiles once)."""
    global _RUNNER
    if _RUNNER is not None:
        return _RUNNER
    import jax
    import concourse.mybir as mybir
    from concourse import bass2jax
    from jax.experimental.shard_map import shard_map
    from jax.sharding import Mesh, PartitionSpec

    nc = build_program()
    bass2jax.install_neuronx_cc_hook()

    partition_name = nc.partition_id_tensor.name if nc.partition_id_tensor else None
    in_names, out_names, out_avals = [], [], []
    for alloc in nc.m.functions[0].allocations:
        if not isinstance(alloc, mybir.MemoryLocationSet):
            continue
        name = alloc.memorylocations[0].name
        if alloc.kind == "ExternalInput":
            if name != partition_name:
                in_names.append(name)
        elif alloc.kind == "ExternalOutput":
            out_names.append(name)
            out_avals.append(
                jax.core.ShapedArray(tuple(alloc.tensor_shape), mybir.dt.np(alloc.dtype)))
    n_params = len(in_names)
    all_names = in_names + out_names
    if partition_name is not None:
        all_names = all_names + [partition_name]
    all_names = tuple(all_names)

    def _body(*args):
        operands = list(args)
        if partition_name is not None:
            operands.append(bass2jax.partition_id_tensor())
        outs = bass2jax._bass_exec_p.bind(
            *operands,
            out_avals=tuple(out_avals),
            in_names=all_names,
            out_names=tuple(out_names),
            lowering_input_output_aliases=(),
            sim_require_finite=True,
            sim_require_nnan=True,
            nc=nc,
        )
        return tuple(outs)

    devices = jax.devices()[:N_CORES]
    mesh = Mesh(np.asarray(devices), ("core",))
    n_outs = len(out_names)
    sharded = jax.jit(
        shard_map(
            _body, mesh=mesh,
            in_specs=(PartitionSpec("core"),) * (n_params + n_outs),
            out_specs=(PartitionSpec("core"),) * n_outs,
            check_rep=False,
        ),
        donate_argnums=tuple(range(n_params, n_params + n_outs)),
        keep_unused=True,
    )

    def run(in_maps):
        concat_in = [
            np.concatenate([np.asarray(in_maps[c][name]) for c in range(N_CORES)], axis=0)
            for name in in_names
        ]
        concat_zeros = [
            np.zeros((N_CORES * a.shape[0], *a.shape[1:]), a.dtype) for a in out_avals
        ]
        out_arrs = sharded(*concat_in, *concat_zeros)
        return [
            {name: np.asarray(out_arrs[i]).reshape(N_CORES, *out_avals[i].shape)[c]
             for i, name in enumerate(out_names)}
            for c in range(N_CORES)
        ]

    _RUNNER = run
    return run


def kernel(x, w_qkv, w_out):
    x = np.asarray(x)
    w_qkv = np.asarray(w_qkv)
    w_out = np.asarray(w_out)
    run = get_runner()
    in_maps = make_in_maps(x, w_qkv, w_out)
    return assemble(run(in_maps))


# revision 10
# speedup vs baseline: 23.0079x; 23.0079x over previous
"""Causal self-attention (B=2, S=2048, D=1024, H=16, hd=64) on 8 TRN2 cores.

Sharding: data-parallel over batch (2) x tensor-parallel over heads (16/4=4
heads per core).  Each core computes qkv projections for its 4 heads, RoPE,
causal flash-attention, and a partial output projection (row-parallel over
the 256 local attention channels).  Host sums the 4 partials per batch.

Numerics: fp16 operands everywhere on the PE (1 cycle/row), fp32 PSUM
accumulation.  Softmax without max-subtraction (scores ~ N(0,1), exp is
safe) so the denominator comes from an all-ones column appended to V.

Layout tricks:
 - Scores are computed transposed (S^T[k, q]) so probabilities feed the
   PV matmul directly as the stationary operand - no PE transposes of P.
 - RoPE pairs are host-permuted to a half-split layout (rotation partner
   lives 32 partitions away); the partner tensor is made with 4 SBUF->SBUF
   partition-swap DMAs and the sign lives in the host-built sin table.
"""

import math
import os

import numpy as np

D_MODEL = 1024
NUM_HEADS = 16
HEAD_DIM = 64
S = 2048
B = 2
N_CORES = 8
HPC = 4  # heads per core
EV = HPC * HEAD_DIM  # 256 local attention channels
ROPE_THETA = 10000.0
KB = S // 128  # 16 key blocks
F16 = np.float16

_PROGRAM = None  # (nc, input names) cache


# --------------------------------------------------------------------------
# host-side input prep
# --------------------------------------------------------------------------

def _rope_rows(base):
    """Row indices of one head's projection in half-split (permuted) order."""
    return [base + 2 * i for i in range(32)] + [base + 2 * i + 1 for i in range(32)]


def _rope_tables():
    inv_freq = 1.0 / (ROPE_THETA ** (np.arange(0, HEAD_DIM, 2, dtype=np.float64) / HEAD_DIM))
    freqs = np.outer(np.arange(S, dtype=np.float64), inv_freq)  # [S, 32]
    cos_t = np.cos(freqs).T  # [32, S]
    sin_t = np.sin(freqs).T
    cos_full = np.tile(cos_t, (4, 1)).astype(F16)  # [128, S]
    sin_full = np.tile(np.concatenate([-sin_t, sin_t], axis=0), (2, 1)).astype(F16)
    return cos_full, sin_full


def _prep_core_inputs(x, w_qkv, w_out, core):
    b, hg = core // 4, core % 4
    heads = [HPC * hg + j for j in range(HPC)]

    xT = np.ascontiguousarray(x[b].T).astype(F16)  # [1024, 2048]

    row_order = []
    for base in (0, D_MODEL):  # q rows then k rows
        for h in heads:
            row_order += _rope_rows(base + h * HEAD_DIM)
    wqk = np.ascontiguousarray(w_qkv[row_order].T).astype(F16)  # [1024, 512]

    v_rows = [2 * D_MODEL + h * HEAD_DIM + j for h in heads for j in range(HEAD_DIM)]
    wv = np.ascontiguousarray(w_qkv[v_rows].T).astype(F16)  # [1024, 256]

    out_cols = [h * HEAD_DIM + j for h in heads for j in range(HEAD_DIM)]
    wout = np.ascontiguousarray(w_out[:, out_cols].T).astype(F16)  # [256, 1024]

    cos_full, sin_full = _rope_tables()
    tri = (np.arange(128)[None, :] >= np.arange(128)[:, None]).astype(F16)  # [k,q] keep q>=k
    ident = np.eye(128, dtype=F16)

    return {
        "xT": xT,
        "wqk": wqk,
        "wv": wv,
        "wout": wout,
        "cos_t": cos_full,
        "sin_t": sin_full,
        "tri": tri,
        "ident": ident,
    }


# --------------------------------------------------------------------------
# device program
# --------------------------------------------------------------------------

def _build_body(tc, io):
    import concourse.bass as bass
    import concourse.mybir as mybir
    from contextlib import ExitStack

    f16 = mybir.dt.float16
    f32 = mybir.dt.float32
    nc = tc.nc

    xT_d, wqk_d, wv_d, wout_d = io["xT"], io["wqk"], io["wv"], io["wout"]
    cos_d, sin_d, tri_d, ident_d, out_d = (
        io["cos_t"], io["sin_t"], io["tri"], io["ident"], io["out"],
    )

    with ExitStack() as ctx:
        const = ctx.enter_context(tc.tile_pool(name="const", bufs=1))
        vpool = ctx.enter_context(tc.tile_pool(name="vpool", bufs=1))
        qkr = ctx.enter_context(tc.tile_pool(name="qkr", bufs=1))
        attn_p = ctx.enter_context(tc.tile_pool(name="attn", bufs=1))

        # ---- persistent constants -------------------------------------
        wqk_sb = [const.tile([128, 512], f16, tag=f"wqk{d}", name=f"wqk{d}") for d in range(8)]
        wv_sb = [const.tile([128, EV], f16, tag=f"wv{d}", name=f"wv{d}") for d in range(8)]
        wout_sb = [const.tile([128, 1024], f16, tag=f"wout{t}", name=f"wout{t}") for t in range(2)]
        cos_sb = const.tile([128, S], f16, tag="cos", name="cos")
        sin_sb = const.tile([128, S], f16, tag="sin", name="sin")
        tri_sb = const.tile([128, 128], f16, tag="tri", name="tri")
        id_sb = const.tile([128, 128], f16, tag="ident", name="ident")
        for d in range(8):
            nc.sync.dma_start(out=wqk_sb[d][:], in_=wqk_d[128 * d:128 * (d + 1), :])
            nc.sync.dma_start(out=wv_sb[d][:], in_=wv_d[128 * d:128 * (d + 1), :])
        for t in range(2):
            nc.sync.dma_start(out=wout_sb[t][:], in_=wout_d[128 * t:128 * (t + 1), :])
        nc.sync.dma_start(out=cos_sb[:], in_=cos_d[:])
        nc.sync.dma_start(out=sin_sb[:], in_=sin_d[:])
        nc.sync.dma_start(out=tri_sb[:], in_=tri_d[:])
        nc.sync.dma_start(out=id_sb[:], in_=ident_d[:])

        # V tiles [128 kpos, 64+1], last col = 1.0 (softmax denominator)
        v_sb = [[vpool.tile([128, HEAD_DIM + 1], f16, tag=f"v{h}_{i}", name=f"v{h}_{i}")
                 for i in range(KB)] for h in range(HPC)]
        for h in range(HPC):
            for i in range(KB):
                nc.gpsimd.memset(v_sb[h][i][:, HEAD_DIM:HEAD_DIM + 1], 1.0)

        # rotated q/k, [128 = 2 heads x 64 dims, S]
        qr_sb = [qkr.tile([128, S], f16, tag=f"qr{t}", name=f"qr{t}") for t in range(2)]
        kr_sb = [qkr.tile([128, S], f16, tag=f"kr{t}", name=f"kr{t}") for t in range(2)]

        # attn^T [128 local dims, S]
        at_sb = [attn_p.tile([128, S], f16, tag=f"at{t}", name=f"at{t}") for t in range(2)]

        with ExitStack() as phase_bc:
            px = phase_bc.enter_context(tc.tile_pool(name="px", bufs=1))
            prope = phase_bc.enter_context(tc.tile_pool(name="prope", bufs=1))
            ptmp = phase_bc.enter_context(tc.tile_pool(name="ptmp", bufs=2))
            ps_big = phase_bc.enter_context(
                tc.tile_pool(name="ps_big", bufs=4, space="PSUM"))

            xT_sb = [px.tile([128, S], f16, tag=f"x{d}", name=f"x{d}") for d in range(8)]
            for d in range(8):
                nc.sync.dma_start(out=xT_sb[d][:], in_=xT_d[128 * d:128 * (d + 1), :])

            # ---- QK projection:  psum[e=128, s=512] = sum_d W^T x^T ----
            q_sb = [prope.tile([128, S], f16, tag=f"q{t}", name=f"q{t}") for t in range(2)]
            k_sb = [prope.tile([128, S], f16, tag=f"k{t}", name=f"k{t}") for t in range(2)]
            qk_dst = q_sb + k_sb
            for t in range(4):
                for sc in range(4):
                    ps = ps_big.tile([128, 512], f32, tag="mm", name="mm")
                    for d in range(8):
                        nc.tensor.matmul(
                            ps[:],
                            lhsT=wqk_sb[d][:, 128 * t:128 * (t + 1)],
                            rhs=xT_sb[d][:, 512 * sc:512 * (sc + 1)],
                            start=(d == 0), stop=(d == 7),
                        )
                    nc.vector.tensor_copy(
                        qk_dst[t][:, 512 * sc:512 * (sc + 1)], ps[:])

            # ---- V projection:  psum[s=128, ev=256] = sum_d x^T^T Wv ----
            for sb in range(KB):
                ps = ps_big.tile([128, EV], f32, tag="mm", name="mm")
                for d in range(8):
                    nc.tensor.matmul(
                        ps[:],
                        lhsT=xT_sb[d][:, 128 * sb:128 * (sb + 1)],
                        rhs=wv_sb[d][:],
                        start=(d == 0), stop=(d == 7),
                    )
                for h in range(HPC):
                    nc.vector.tensor_copy(
                        v_sb[h][sb][:, 0:HEAD_DIM],
                        ps[:, HEAD_DIM * h:HEAD_DIM * (h + 1)])

            # ---- RoPE ------------------------------------------------
            # qr = q*cos + swap32(q)*sin_signed   (sign folded into table)
            for src, dst in zip(q_sb + k_sb, qr_sb + kr_sb):
                sw = ptmp.tile([128, S], f16, tag="sw", name="sw")
                for a, bq in ((0, 32), (32, 0), (64, 96), (96, 64)):
                    nc.sync.dma_start(out=sw[a:a + 32, :], in_=src[bq:bq + 32, :])
                t1 = ptmp.tile([128, S], f16, tag="t1", name="t1")
                nc.vector.tensor_mul(t1[:], src[:], cos_sb[:])
                t2 = ptmp.tile([128, S], f16, tag="t2", name="t2")
                nc.vector.tensor_mul(t2[:], sw[:], sin_sb[:])
                nc.vector.tensor_add(dst[:], t1[:], t2[:])

        # ---- attention + output projection ---------------------------
        with ExitStack() as phase_de:
            epool = phase_de.enter_context(tc.tile_pool(name="epool", bufs=18))
            pstage = phase_de.enter_context(tc.tile_pool(name="pstage", bufs=4))
            pnorm = phase_de.enter_context(tc.tile_pool(name="pnorm", bufs=4))
            ps_s = phase_de.enter_context(
                tc.tile_pool(name="ps_s", bufs=2, space="PSUM"))
            ps_sm = phase_de.enter_context(
                tc.tile_pool(name="ps_sm", bufs=2, space="PSUM"))

            scale = 1.0 / math.sqrt(HEAD_DIM)
            for h in range(HPC):
                tq, ro = h // 2, (h % 2) * 64
                e_tiles = []
                # scores^T + exp, k-block at a time
                for i in range(KB):
                    q_lo = 128 * i
                    et = epool.tile([128, S], f16, tag="e", name="e")
                    e_tiles.append(et)
                    for qc in range(q_lo, S, 1024):
                        qw = min(1024, S - qc)
                        ps = ps_s.tile([128, 1024], f32, tag="s", name="s")
                        for o in range(0, qw, 512):
                            w = min(512, qw - o)
                            nc.tensor.matmul(
                                ps[:, o:o + w],
                                lhsT=kr_sb[tq][ro:ro + 64, q_lo:q_lo + 128],
                                rhs=qr_sb[tq][ro:ro + 64, qc + o:qc + o + w],
                                start=True, stop=True,
                            )
                        nc.scalar.activation(
                            et[:, qc:qc + qw], ps[:, 0:qw],
                            mybir.ActivationFunctionType.Exp, scale=scale)
                    # causal mask of the diagonal block
                    nc.vector.tensor_mul(
                        et[:, q_lo:q_lo + 128], et[:, q_lo:q_lo + 128], tri_sb[:])

                # PV + normalize + transpose into attn^T
                for bq in range(KB):
                    pav = ps_sm.tile([128, HEAD_DIM + 1], f32, tag="av", name="av")
                    for i in range(bq + 1):
                        nc.tensor.matmul(
                            pav[:],
                            lhsT=e_tiles[i][:, 128 * bq:128 * (bq + 1)],
                            rhs=v_sb[h][i][:],
                            start=(i == 0), stop=(i == bq),
                        )
                    rec = pnorm.tile([128, 1], f32, tag="rec", name="rec")
                    nc.vector.reciprocal(rec[:], pav[:, HEAD_DIM:HEAD_DIM + 1])
                    ab = pnorm.tile([128, HEAD_DIM], f16, tag="ab", name="ab")
                    nc.vector.tensor_scalar_mul(ab[:], pav[:, 0:HEAD_DIM], rec[:])
                    ptr = ps_sm.tile([64, 128], f16, tag="tr", name="tr")
                    nc.tensor.transpose(ptr[:], ab[:], id_sb[:])
                    nc.vector.tensor_copy(
                        at_sb[tq][ro:ro + 64, 128 * bq:128 * (bq + 1)], ptr[:])

            # ---- output projection ----------------------------------
            for sb in range(KB):
                for ec in range(2):
                    ps = ps_s.tile([128, 512], f32, tag="s", name="s")
                    for t in range(2):
                        nc.tensor.matmul(
                            ps[:],
                            lhsT=at_sb[t][:, 128 * sb:128 * (sb + 1)],
                            rhs=wout_sb[t][:, 512 * ec:512 * (ec + 1)],
                            start=(t == 0), stop=(t == 1),
                        )
                    ot = pstage.tile([128, 512], f16, tag="o", name="o")
                    nc.vector.tensor_copy(ot[:], ps[:])
                    nc.sync.dma_start(
                        out=out_d[128 * sb:128 * (sb + 1), 512 * ec:512 * (ec + 1)],
                        in_=ot[:])


def build_program():
    global _PROGRAM
    if _PROGRAM is not None:
        return _PROGRAM
    import concourse.bacc as bacc
    import concourse.tile as tile
    import concourse.mybir as mybir

    f16 = mybir.dt.float16
    nc = bacc.Bacc("TRN2", target_bir_lowering=False, debug=False)
    io = {
        "xT": nc.dram_tensor("xT", [D_MODEL, S], f16, kind="ExternalInput").ap(),
        "wqk": nc.dram_tensor("wqk", [D_MODEL, 512], f16, kind="ExternalInput").ap(),
        "wv": nc.dram_tensor("wv", [D_MODEL, EV], f16, kind="ExternalInput").ap(),
        "wout": nc.dram_tensor("wout", [EV, D_MODEL], f16, kind="ExternalInput").ap(),
        "cos_t": nc.dram_tensor("cos_t", [128, S], f16, kind="ExternalInput").ap(),
        "sin_t": nc.dram_tensor("sin_t", [128, S], f16, kind="ExternalInput").ap(),
        "tri": nc.dram_tensor("tri", [128, 128], f16, kind="ExternalInput").ap(),
        "ident": nc.dram_tensor("ident", [128, 128], f16, kind="ExternalInput").ap(),
        "out": nc.dram_tensor("out", [S, D_MODEL], f16, kind="ExternalOutput").ap(),
    }
    with tile.TileContext(nc) as tc:
        _build_body(tc, io)
    nc.compile()
    _PROGRAM = nc
    return nc


def make_in_maps(x, w_qkv, w_out):
    return [
        {k: v for k, v in _prep_core_inputs(x, w_qkv, w_out, c).items()}
        for c in range(N_CORES)
    ]


def assemble(results):
    """results: list of 8 dicts with 'out' [S, D] fp16 -> full [B, S, D] fp32."""
    out = np.zeros((B, S, D_MODEL), dtype=np.float32)
    for c in range(N_CORES):
        out[c // 4] += results[c]["out"].astype(np.float32)
    return out


_RUNNER = None


def get_runner():
    """Persistent jitted shard_map over the 8 cores (compiles once)."""
    global _RUNNER
    if _RUNNER is not None:
        return _RUNNER
    import jax
    import concourse.mybir as mybir
    from concourse import bass2jax
    from jax.experimental.shard_map import shard_map
    from jax.sharding import Mesh, PartitionSpec

    nc = build_program()
    bass2jax.install_neuronx_cc_hook()

    partition_name = nc.partition_id_tensor.name if nc.partition_id_tensor else None
    in_names, out_names, out_avals = [], [], []
    for alloc in nc.m.functions[0].allocations:
        if not isinstance(alloc, mybir.MemoryLocationSet):
            continue
        name = alloc.memorylocations[0].name
        if alloc.kind == "ExternalInput":
            if name != partition_name:
                in_names.append(name)
        elif alloc.kind == "ExternalOutput":
            out_names.append(name)
            out_avals.append(
                jax.core.ShapedArray(tuple(alloc.tensor_shape), mybir.dt.np(alloc.dtype)))
    n_params = len(in_names)
    all_names = in_names + out_names
    if partition_name is not None:
        all_names = all_names + [partition_name]
    all_names = tuple(all_names)

    def _body(*args):
        operands = list(args)
        if partition_name is not None:
            operands.append(bass2jax.partition_id_tensor())
        outs = bass2jax._bass_exec_p.bind(
            *operands,
            out_avals=tuple(out_avals),
            in_names=all_names,
            out_names=tuple(out_names),
            lowering_input_output_aliases=(),
            sim_require_finite=True,
            sim_require_nnan=True,
            nc=nc,
        )
        return tuple(outs)

    devices = jax.devices()[:N_CORES]
    mesh = Mesh(np.asarray(devices), ("core",))
    n_outs = len(out_names)
    # no donation: the kernel writes every output element, so the zero
    # "initial output" buffers stay valid and are reused across calls.
    sharded = jax.jit(
        shard_map(
            _body, mesh=mesh,
            in_specs=(PartitionSpec("core"),) * (n_params + n_outs),
            out_specs=(PartitionSpec("core"),) * n_outs,
            check_rep=False,
        ),
        keep_unused=True,
    )

    from jax.sharding import NamedSharding

    shard = NamedSharding(mesh, PartitionSpec("core"))
    zero_shapes = [(N_CORES * a.shape[0], *a.shape[1:]) for a in out_avals]
    zero_dtypes = [a.dtype for a in out_avals]

    _zeros_cache = []

    def _make_zeros():
        if not _zeros_cache:
            _zeros_cache.append(tuple(
                jax.device_put(np.zeros(s, d), shard)
                for s, d in zip(zero_shapes, zero_dtypes)
            ))
        return _zeros_cache[0]

    def place_inputs(in_maps):
        concat_in = [
            np.concatenate([np.asarray(in_maps[c][name]) for c in range(N_CORES)], axis=0)
            for name in in_names
        ]
        return [jax.device_put(a, shard) for a in concat_in]

    def call(placed):
        zeros = _make_zeros()
        return sharded(*placed, *zeros)

    def fetch(out_arrs):
        return [
            {name: np.asarray(out_arrs[i]).reshape(N_CORES, *out_avals[i].shape)[c]
             for i, name in enumerate(out_names)}
            for c in range(N_CORES)
        ]

    def run(in_maps):
        return fetch(call(place_inputs(in_maps)))

    run.place_inputs = place_inputs
    run.call = call
    run.fetch = fetch
    _RUNNER = run
    return run


def kernel(x, w_qkv, w_out):
    x = np.asarray(x)
    w_qkv = np.asarray(w_qkv)
    w_out = np.asarray(w_out)
    run = get_runner()
    in_maps = make_in_maps(x, w_qkv, w_out)
    return assemble(run(in_maps))
